# revision 1
# baseline (speedup 1.0000x reference)
"""AdderNet BasicBlock (Adder2D 3x3 + BatchNorm(train) + ReLU) on 8 TRN2 cores.

Problem: x[4,64,32,32], weight[64,64,3,3], gamma[64], beta[64] ->
    out[b,o,y,x] = relu(BN_train(-sum_{c,ky,kx} |x_pad[b,c,y+ky,x+kx] - w[o,c,ky,kx]|))

Sharding: output channels O=64 split 8 per core. BatchNorm stats are per-channel
over (B,H,W), so each core's 8 channels are fully self-contained: no collectives.

Per-core dataflow (all shapes hardcoded):
  - x held in SBUF as XP[128, 2*34*34] f32: partition p = (h, c) with h=p//64,
    c=p%64; free (u, y, x) holds batch b = 2*u + h, zero-padded spatial. One
    strided view covers all 4 batches at F=2048 per (output-channel o, tap).
    f16 copies (plus a 1-element-shifted one for odd tap offsets, keeping APs
    4-byte aligned) feed the DVE 4x-mode path.
  - For each (o, tap): D[128, 2, 32, 32] f16 = |XP_view - w[o, c, tap]|:
    ~30 taps on ACT (activation Abs with per-partition bias=-w, one fused op),
    the rest on DVE (tensor_scalar subtract at 4x, then sign-bit clear via
    tensor_scalar bitwise_and 0x7FFF on the u16 view, also 4x). abs_max and
    tensor_tensor_reduce are rejected/broken in HW ISA; this chain is exact.
  - PE reduces over partitions, accumulating all 8 channels x 9 taps into one
    persistent PSUM tile S[32, 1024]. lhsT is a one-hot f16 selector
    msel(o,u)[128, 32] mapping partition half h to output row o*4 + 2u + h, so
    every matmul writes base partition 0 (hardware constraint) and rows for
    other channels just accumulate zeros. f16 matmuls measured 221.6 ns at
    N=512 (f32r pays +45ns/matmul on weight load; fp32 is 4x slower).
  - Epilogue from PSUM: per-channel mean via free-reduce + tiny selector matmul;
    centered subtract (doubles as the PSUM drain), ACT Square+accum for var,
    out = relu((S-m)*A + beta), single DMA out [32,1024] = (o_local*4+b, y*32+x).
  - Measured on HW: main loop ~66-70 us/core, plus ~29 us fixed (input DMA,
    BN epilogue, Tile tail barrier); rel RMS error ~5e-5 (f16 D quantization).

kernel() is self-contained: builds the Bass program once, shards inputs on host,
runs via bass_utils.run_bass_kernel_spmd on cores 0..7, reassembles full output.
"""

import functools
import os

import numpy as np

B, C, O, H, W = 4, 64, 64, 32, 32
K, PAD = 3, 1
HP, WP = H + 2 * PAD, W + 2 * PAD  # 34, 34
L = H * W  # 1024
SPP = HP * WP  # 1156 padded spatial per batch
NCORES = 8
O_PER = O // NCORES  # 8
NB2 = B // 2  # bpairs
EPS = 1e-5
NSTAT = O_PER * B  # 32 rows of S
NPIX = B * L  # 4096 values per channel for BN stats

# absdiff engine split: 72 ops per core (8 o x 9 taps). ACT is a bit faster per op
# ((F+222)*0.83 vs (F+58)*1.04 ns), so give it more.
N_ACT_OPS = int(os.environ.get("KRN_ACT_OPS", "32"))
N_GPS_OPS = int(os.environ.get("KRN_GPS_OPS", "0"))  # gpsimd subtract is ~10x too slow on HW
N_PRESUM = int(os.environ.get("KRN_PRESUM", "0"))  # only pays once producers beat PE
D_BUFS = int(os.environ.get("KRN_D_BUFS", "10"))
MM_REPEAT = int(os.environ.get("KRN_MM_REPEAT", "1"))  # bench-only: scales PE work


def _engine_schedule(n_ops: int):
    """Return list of 'v'/'a'/'g' of length n_ops, interleaving engines evenly."""
    n_g = min(N_GPS_OPS, n_ops)
    n_a = min(N_ACT_OPS, n_ops - n_g)
    n_v = n_ops - n_a - n_g
    counts = {"v": n_v, "a": n_a, "g": n_g}
    acc = {k: 0.0 for k in counts}
    sched = []
    for _ in range(n_ops):
        for k in counts:
            acc[k] += counts[k] / n_ops
        pick = max(acc, key=lambda k: acc[k])
        acc[pick] -= 1.0
        sched.append(pick)
    return sched




def _emit_main(nc, tc, mybir, xp4, xph4, xpho4, wcols, nwcols, mselh, dpool, ps, sched):
    from concourse import mybir as _mb

    f16 = _mb.dt.float16
    u16 = _mb.dt.uint16

    def absdiff(o, tap):
        """Emit |x - w| for (o, tap) -> f16 tile d[128, NB2, H, W]."""
        ky, kx = tap // 3, tap % 3
        idx = o * 9 + tap
        eng = sched[idx]
        if eng == "a":
            view = xp4[:, :, ky : ky + H, kx : kx + W]
            d = dpool.tile([128, NB2, H, W], f16, tag="da", name=f"da{idx}")
            nc.scalar.activation(
                out=d[:], in_=view, func=_mb.ActivationFunctionType.Abs,
                bias=nwcols[:, idx : idx + 1], scale=1.0,
            )
            return d
        if kx == 1:
            view = xpho4[:, :, ky : ky + H, kx - 1 : kx - 1 + W]
        else:
            view = xph4[:, :, ky : ky + H, kx : kx + W]
        d1 = dpool.tile([128, NB2, H, W], f16, tag="d1", name=f"d1_{idx}")
        if eng == "g":
            nc.gpsimd.tensor_scalar_sub(d1[:], view, wcols[:, idx : idx + 1])
        else:
            nc.vector.tensor_scalar_sub(d1[:], view, wcols[:, idx : idx + 1])
        d = dpool.tile([128, NB2, H, W], f16, tag="dv", name=f"dv{idx}")
        nc.vector.tensor_scalar(
            out=d[:].bitcast(u16), in0=d1[:].bitcast(u16),
            scalar1=0x7FFF, scalar2=None, op0=_mb.AluOpType.bitwise_and,
        )
        return d

    first = [True, True]

    def mm(o, d, last):
        d2 = d.rearrange("p u a b -> p (u a b)")
        for rep in range(MM_REPEAT):
            for u in range(NB2):
                for half in range(2):
                    nc.tensor.matmul(
                        ps[half][:, :],
                        lhsT=mselh[:, (o * 2 + u) * NSTAT : (o * 2 + u + 1) * NSTAT],
                        rhs=d2[:, u * L + half * 512 : u * L + half * 512 + 512],
                        start=first[half],
                        stop=(last and rep == MM_REPEAT - 1 and u == NB2 - 1),
                    )
                    first[half] = False

    for o in range(O_PER):
        taps = list(range(9))
        # pick one presum pair per channel for the first N_PRESUM channels:
        # two non-ACT taps whose |diff| tiles get added on DVE before PE.
        pair = None
        if o < N_PRESUM:
            cand = [t for t in taps if sched[o * 9 + t] != "a"]
            if len(cand) >= 2:
                pair = (cand[0], cand[1])
        last_of_o = o == O_PER - 1
        if pair is not None:
            di = absdiff(o, pair[0])
            dj = absdiff(o, pair[1])
            dsum = dpool.tile([128, NB2, H, W], f16, tag="dsum", name=f"dsum{o}")
            nc.vector.tensor_add(dsum[:], di[:], dj[:])
            mm(o, dsum, False)
        rest = [t for t in taps if pair is None or t not in pair]
        for i, tap in enumerate(rest):
            d = absdiff(o, tap)
            mm(o, d, last_of_o and i == len(rest) - 1)

@functools.lru_cache(maxsize=4)
def _build_program(bench_iters=0):
    from contextlib import ExitStack

    import concourse.tile as tile
    from concourse import bacc, mybir

    f32 = mybir.dt.float32
    f32r = mybir.dt.float32r
    f16 = mybir.dt.float16
    u16 = mybir.dt.uint16

    nc = bacc.Bacc("TRN2", target_bir_lowering=False, debug=False)

    x_t = nc.dram_tensor("x", (B, C, H, W), f32, kind="ExternalInput")
    # wpack[:, :72] = wcols (w[o_g, p%64, tap]), [:, 72:144] = -wcols
    wpack_t = nc.dram_tensor("wpack", (128, 2 * O_PER * 9), f32, kind="ExternalInput")
    # mselh[p, (o*2+u)*32 + j] = 1.0 iff j == o*4 + 2u + p//64
    mselh_t = nc.dram_tensor("mselh", (128, O_PER * 2 * NSTAT), f16, kind="ExternalInput")
    # spack[:, :32] = osel, [:, 32] = -gamma col, [:, 33] = beta col
    spack_t = nc.dram_tensor("spack", (NSTAT, NSTAT + 2), f32, kind="ExternalInput")
    out_t = nc.dram_tensor("out", (NSTAT, L), f32, kind="ExternalOutput")

    sched = _engine_schedule(O_PER * 9)

    with tile.TileContext(nc) as tc, ExitStack() as ctx:
        consts = ctx.enter_context(tc.tile_pool(name="consts", bufs=1))
        dpool = ctx.enter_context(tc.tile_pool(name="dpool", bufs=D_BUFS))
        spool = ctx.enter_context(tc.tile_pool(name="spool", bufs=2))
        psum_main = ctx.enter_context(tc.tile_pool(name="psum_main", bufs=1, space="PSUM"))
        psum_stat = ctx.enter_context(tc.tile_pool(name="psum_stat", bufs=2, space="PSUM"))

        # ---- constants / inputs to SBUF ----
        wpack = consts.tile([128, 2 * O_PER * 9], f32)
        mselh = consts.tile([128, O_PER * 2 * NSTAT], f16)
        spack = consts.tile([NSTAT, NSTAT + 2], f32)
        nc.sync.dma_start(out=wpack[:], in_=wpack_t[:, :])
        nc.sync.dma_start(out=mselh[:], in_=mselh_t[:, :])
        nc.sync.dma_start(out=spack[:], in_=spack_t[:, :])
        wcols = wpack[:, 0 : O_PER * 9]
        nwcols = wpack[:, O_PER * 9 : 2 * O_PER * 9]
        osel = spack[:, 0:NSTAT]
        gcol = spack[:, NSTAT : NSTAT + 1]
        bcol = spack[:, NSTAT + 1 : NSTAT + 2]

        # ---- padded input: XP[128, 2*1156], partition=(b_half, c), free=(bpair, y, x)
        xp = consts.tile([128, NB2 * SPP], f32)
        xp4 = xp.rearrange("p (u a b) -> p u a b", u=NB2, a=HP, b=WP)
        for u in range(NB2):
            nc.gpsimd.memset(xp4[:, u, 0, :], 0.0)
            nc.gpsimd.memset(xp4[:, u, HP - 1, :], 0.0)
            nc.gpsimd.memset(xp4[:, u, :, 0], 0.0)
            nc.gpsimd.memset(xp4[:, u, :, WP - 1], 0.0)
        for b in range(B):
            h, u = b % 2, b // 2
            nc.sync.dma_start(
                out=xp4[h * 64 : h * 64 + 64, u, PAD : PAD + H, PAD : PAD + W],
                in_=x_t[b, :, :, :],
            )

        # f16 copy of the padded input for the 4x-mode DVE chain
        xph = consts.tile([128, NB2 * SPP], f16)
        nc.vector.tensor_copy(out=xph[:], in_=xp[:])
        xph4 = xph.rearrange("p (u a b) -> p u a b", u=NB2, a=HP, b=WP)
        # +1-element shifted copy: keeps the kx==1 taps 4-byte aligned for 4x mode
        xpho = consts.tile([128, NB2 * SPP], f16)
        nc.scalar.copy(out=xpho[:, 0 : NB2 * SPP - 1], in_=xph[:, 1 : NB2 * SPP])
        xpho4 = xpho.rearrange("p (u a b) -> p u a b", u=NB2, a=HP, b=WP)

        # ---- main loop: S[o*4+b, l] accumulates over taps in two PSUM halves ----
        ps_big = psum_main.tile([NSTAT, 2 * 512], f32, name="ps_big")
        ps = [ps_big[:, h * 512 : h * 512 + 512] for h in range(2)]
        import contextlib

        loop_cm = (
            tc.For_i(0, bench_iters, 1) if bench_iters else contextlib.nullcontext()
        )
        with loop_cm:
            _emit_main(nc, tc, mybir, xp4, xph4, xpho4, wcols, nwcols, mselh, dpool, ps, sched)

        # ---- epilogue: BN stats + normalize + relu ----
        # per-row sums over l, then per-channel (replicated) via selector matmul
        sums = spool.tile([NSTAT, 1], f32, tag="small1")
        nc.vector.tensor_reduce(
            out=sums[:], in_=ps_big[:], axis=mybir.AxisListType.X,
            op=mybir.AluOpType.add,
        )
        sum_ps = psum_stat.tile([NSTAT, 1], f32, tag="statps")
        nc.tensor.matmul(
            sum_ps[:], lhsT=osel, rhs=sums[:],
            start=True, stop=True,
        )
        m32 = spool.tile([NSTAT, 1], f32, tag="small2")
        nc.vector.tensor_scalar_mul(m32[:], sum_ps[:], 1.0 / NPIX)

        # centered values (also drains PSUM -> SBUF)
        dctr = spool.tile([NSTAT, L], f32, tag="big")
        nc.vector.tensor_scalar_sub(dctr[:], ps_big[:], m32[:])
        scr = spool.tile([NSTAT, L], f32, tag="big2")
        sqs = spool.tile([NSTAT, 1], f32, tag="small3")
        nc.scalar.activation(
            out=scr[:], in_=dctr[:], func=mybir.ActivationFunctionType.Square,
            accum_out=sqs[:],
        )
        var_ps = psum_stat.tile([NSTAT, 1], f32, tag="statps2")
        nc.tensor.matmul(
            var_ps[:], lhsT=osel, rhs=sqs[:],
            start=True, stop=True,
        )
        # std = sqrt(var/NPIX + eps); rinv = 1/std
        epscol = spool.tile([NSTAT, 1], f32, tag="eps")
        nc.vector.memset(epscol[:], EPS)
        std32 = spool.tile([NSTAT, 1], f32, tag="small4")
        nc.scalar.activation(
            out=std32[:], in_=var_ps[:], func=mybir.ActivationFunctionType.Sqrt,
            bias=epscol[:], scale=1.0 / NPIX,
        )
        rinv = spool.tile([NSTAT, 1], f32, tag="small5")
        nc.vector.reciprocal(rinv[:], std32[:])
        # A = -gamma*rinv ; out = relu((S - m)*A + beta)
        acol = spool.tile([NSTAT, 1], f32, tag="small7")
        nc.vector.tensor_mul(acol[:], gcol, rinv[:])

        outf = spool.tile([NSTAT, L], f32, tag="outf")
        nc.scalar.activation(
            out=outf[:], in_=dctr[:], func=mybir.ActivationFunctionType.Relu,
            bias=bcol, scale=acol[:],
        )
        nc.sync.dma_start(out=out_t[:, :], in_=outf[:])

    nc.compile()
    return nc


def _host_inputs(x, weight, gamma, beta):
    """Build the 8 per-core input maps."""
    x = np.ascontiguousarray(x, dtype=np.float32)
    weight = np.asarray(weight, dtype=np.float32)
    gamma = np.asarray(gamma, dtype=np.float32)
    beta = np.asarray(beta, dtype=np.float32)

    msel = np.zeros((128, O_PER * 2 * NSTAT), dtype=np.float32)
    for o in range(O_PER):
        for u in range(NB2):
            for p_half in range(2):
                j = o * 4 + 2 * u + p_half
                col = (o * 2 + u) * NSTAT + j
                msel[p_half * 64 : (p_half + 1) * 64, col] = 1.0
    osel = np.zeros((NSTAT, NSTAT), dtype=np.float32)
    for p in range(NSTAT):
        for m in range(NSTAT):
            if p // B == m // B:
                osel[p, m] = 1.0

    in_maps = []
    for core in range(NCORES):
        osl = slice(core * O_PER, (core + 1) * O_PER)
        w = weight[osl]  # [8, 64, 3, 3]
        # wcols[p, o*9+tap] = w[o, p%64, tap//3, tap%3]
        wc = w.reshape(O_PER, C, 9).transpose(1, 0, 2).reshape(C, O_PER * 9)
        wcols = np.concatenate([wc, wc], axis=0).astype(np.float32)  # [128, 72]
        wpack = np.concatenate([wcols, -wcols], axis=1)  # [128, 144]
        # gcol[p] = -gamma[o(p)] with o = p//4 (A = -gamma*rinv)
        gcol = np.repeat(-gamma[osl], B).reshape(NSTAT, 1).astype(np.float32)
        bcol = np.repeat(beta[osl], B).reshape(NSTAT, 1).astype(np.float32)
        spack = np.concatenate([osel, gcol, bcol], axis=1)  # [32, 34]
        in_maps.append(
            {
                "x": x,
                "wpack": np.ascontiguousarray(wpack),
                "mselh": msel.astype(np.float16),
                "spack": np.ascontiguousarray(spack),
            }
        )
    return in_maps


def _assemble(results):
    out = np.empty((B, O, H, W), dtype=np.float32)
    for core, res in enumerate(results):
        arr = res["out"].reshape(O_PER, B, H, W)  # row = o*4+b
        out[:, core * O_PER : (core + 1) * O_PER] = arr.transpose(1, 0, 2, 3)
    return out


def kernel(x, weight, gamma, beta, _trace=False):
    from concourse import bass_utils

    nc = _build_program()
    in_maps = _host_inputs(x, weight, gamma, beta)
    res = bass_utils.run_bass_kernel_spmd(
        nc, in_maps, core_ids=list(range(NCORES)), trace=_trace
    )
    out = _assemble(res.results)
    if _trace:
        return out, res
    return out



# revision 20
# speedup vs baseline: 1.0816x; 1.0816x over previous
"""AdderNet BasicBlock (Adder2D 3x3 + BatchNorm(train) + ReLU) on 8 TRN2 cores.

Problem: x[4,64,32,32], weight[64,64,3,3], gamma[64], beta[64] ->
    out[b,o,y,x] = relu(BN_train(-sum_{c,ky,kx} |x_pad[b,c,y+ky,x+kx] - w[o,c,ky,kx]|))

Sharding: output channels O=64 split 8 per core. BatchNorm stats are per-channel
over (B,H,W), so each core's 8 channels are fully self-contained: no collectives.

Per-core dataflow (all shapes hardcoded):
  - Host supplies XPH[128, 2*34*34] f16: padded input, partition p = (h, c) with
    h=p//64, c=p%64; free (u, y, x) holds batch b = 2*u + h. No on-chip
    conversion or memset; one strided f16 view per (o, tap).
  - ACT taps (kx==1 plus extras): one fused op, activation Abs with
    per-partition bias=-w. All DVE views then have even element offsets
    (kx in {0,2}), keeping 4-byte alignment for DVE 4x mode with no shifted
    input copy.
  - DVE taps: tensor_scalar subtract at 4x into slots of a per-o wide tile,
    then ONE merged sign-bit clear (tensor_scalar bitwise_and 0x7FFF on the
    u16 view, 4x) over all slots — fused sub+and / abs_max op1 variants are
    rejected by the walrus BIR verifier ("mismatch op0(arith) op1(bitwise)").
  - Presum: pairs of |diff| tiles get tensor_tensor-added on DVE (2x mode)
    before PE, trading ~1.1us DVE for ~0.9us PE per pair (PE is the
    bottleneck engine otherwise).
  - PE reduces over partitions, accumulating all 8 channels x 9 taps into one
    persistent PSUM tile S[32, 1024] via one-hot f16 selector lhsT
    (rows o*4 + 2u + h). f16 matmuls ~221.6 ns at N=512.
  - Epilogue from PSUM: mean via free-reduce + selector matmul; variance via
    ACT Square with bias=-mean (centering folded into the activation, no
    separate DVE pass) + accum_out; out = relu(S*A + B) with A = -gamma/std,
    B = beta + mean*gamma/std folded into the final ACT op; single DMA out.

kernel() is self-contained: builds the Bass program once, shards inputs on host,
runs via bass_utils.run_bass_kernel_spmd on cores 0..7, reassembles full output.
"""

import functools
import os

import numpy as np

B, C, O, H, W = 4, 64, 64, 32, 32
K, PAD = 3, 1
HP, WP = H + 2 * PAD, W + 2 * PAD  # 34, 34
L = H * W  # 1024
SPP = HP * WP  # 1156 padded spatial per batch
NCORES = 8
O_PER = O // NCORES  # 8
NB2 = B // 2  # bpairs
EPS = 1e-5
NSTAT = O_PER * B  # 32 rows of S
NPIX = B * L  # 4096 values per channel for BN stats

# ACT taps per core: kx==1 taps not taken by Pool, plus tap-0/tap-3 extras.
N_ACT_OPS = int(os.environ.get("KRN_ACT_OPS", "30"))
# Pool/gpsimd engine taps: measured ~40us+/op on HW (the Pool engine also
# serves DMA descriptor generation) — keep at 0.
N_GPS_OPS = int(os.environ.get("KRN_GPS_OPS", "0"))
N_PRESUM = int(os.environ.get("KRN_PRESUM", "8"))
D_BUFS = int(os.environ.get("KRN_D_BUFS", "8"))
MM_REPEAT = int(os.environ.get("KRN_MM_REPEAT", "1"))  # bench-only: scales PE work
# WIDE=1: DVE subs cover the full 34-wide padded rows (fully contiguous APs,
# guaranteed 4x eligibility) and the PE reads a strided window per kx.
WIDE = int(os.environ.get("KRN_WIDE", "0"))


def _schedule():
    """Per-o tap assignment: (act, dve, pool taps) and global presum pairs."""
    assert N_GPS_OPS <= 16
    # Pool poaches kx==1 taps (tap 4 of each o, then tap 1)
    pool_list = [(o, 4) for o in range(O_PER)] + [(o, 1) for o in range(O_PER)]
    pool_set = set(pool_list[:N_GPS_OPS])
    # ACT: remaining kx==1 taps + extras (tap 0, then tap 3) up to N_ACT_OPS
    extras = [(o, 0) for o in range(O_PER)] + [(o, 3) for o in range(O_PER)]
    n_base = 24 - len(pool_set)
    n_extra = max(0, min(N_ACT_OPS - n_base, len(extras)))
    act_set = {(o, t) for o in range(O_PER) for t in (1, 4, 7)
               if (o, t) not in pool_set}
    act_set |= set(extras[:n_extra])
    plans = []
    for o in range(O_PER):
        act = [t for t in (0, 1, 3, 4, 7) if (o, t) in act_set]
        pool = [t for t in (4, 1) if (o, t) in pool_set]
        dve = [t for t in range(9) if t not in act and t not in pool]
        plans.append((act, dve, pool))
    # presum pair priority: (3,5) for each o, then (6,8), then (0,2) if on DVE
    pairs = []
    for pr in [(3, 5), (6, 8), (0, 2)]:
        for o in range(O_PER):
            if pr[0] in plans[o][1] and pr[1] in plans[o][1]:
                pairs.append((o, pr))
    return plans, pairs[: max(0, min(N_PRESUM, len(pairs)))]


def _emit_main(nc, tc, mybir, xph4, wcols, nwcols, mselh, apool, dpool, spool,
               gpool, ps):
    from concourse import mybir as _mb

    f16 = _mb.dt.float16
    u16 = _mb.dt.uint16
    plans, pairs = _schedule()
    pair_of = {o: [] for o in range(O_PER)}
    for o, pr in pairs:
        pair_of[o].append(pr)

    first = [True, True]
    WD = WP if WIDE else W

    def mm(o, view_fn, last):
        # view_fn(u, half) -> [128, 16, W] rhs window (512 cols)
        for rep in range(MM_REPEAT):
            for u in range(NB2):
                for half in range(2):
                    nc.tensor.matmul(
                        ps[half][:, :],
                        lhsT=mselh[:, (o * 2 + u) * NSTAT : (o * 2 + u + 1) * NSTAT],
                        rhs=view_fn(u, half),
                        start=first[half],
                        stop=(last and rep == MM_REPEAT - 1 and u == NB2 - 1),
                    )
                    first[half] = False

    def tile_view(t):
        return lambda u, half: t[:, u, half * 16 : half * 16 + 16, 0:W]

    def slot_view(t, s, kx0):
        return lambda u, half: t[:, s, u, half * 16 : half * 16 + 16, kx0 : kx0 + W]

    for o in range(O_PER):
        act_taps, dve_taps, pool_taps = plans[o]
        n = len(dve_taps)
        prs = pair_of[o]
        in_pair = {t for pr in prs for t in pr}
        last_of_o = o == O_PER - 1

        # Pool taps (slow 1x engine, otherwise idle): subtract on Pool, then a
        # 4x sign-clear on DVE (walrus rejects bitwise tensor_scalar on Pool)
        pools = []
        for t in pool_taps:
            ky, kx = t // 3, t % 3
            idx = o * 9 + t
            g1 = gpool.tile([128, NB2, H, W], f16, tag="g1", name=f"g1_{idx}")
            nc.gpsimd.tensor_scalar_sub(
                g1[:], xph4[:, :, ky : ky + H, kx : kx + W],
                wcols[:, idx : idx + 1])
            g2 = gpool.tile([128, NB2, H, W], f16, tag="g2", name=f"g2_{idx}")
            nc.vector.tensor_scalar(
                out=g2[:].bitcast(u16), in0=g1[:].bitcast(u16),
                scalar1=0x7FFF, scalar2=None, op0=_mb.AluOpType.bitwise_and)
            pools.append(g2)

        # ACT taps: one fused |x - w| op each
        acts = []
        for t in act_taps:
            ky, kx = t // 3, t % 3
            idx = o * 9 + t
            da = apool.tile([128, NB2, H, W], f16, tag="da", name=f"da{idx}")
            nc.scalar.activation(
                out=da[:], in_=xph4[:, :, ky : ky + H, kx : kx + W],
                func=_mb.ActivationFunctionType.Abs,
                bias=nwcols[:, idx : idx + 1], scale=1.0,
            )
            acts.append(da)

        # DVE taps: subs into slots of one wide tile, one merged AND over all
        dd = dpool.tile([128, 6, NB2, H, WD], f16, tag="dd", name=f"dd{o}")
        ddo = dpool.tile([128, 6, NB2, H, WD], f16, tag="ddo", name=f"ddo{o}")
        slot = {t: i for i, t in enumerate(dve_taps)}
        for t in dve_taps:
            ky, kx = t // 3, t % 3
            idx = o * 9 + t
            src = (xph4[:, :, ky : ky + H, :] if WIDE
                   else xph4[:, :, ky : ky + H, kx : kx + W])
            nc.vector.tensor_scalar_sub(dd[:, slot[t]], src,
                                        wcols[:, idx : idx + 1])
        nc.vector.tensor_scalar(
            out=ddo[:, 0:n].bitcast(u16), in0=dd[:, 0:n].bitcast(u16),
            scalar1=0x7FFF, scalar2=None, op0=_mb.AluOpType.bitwise_and,
        )

        # presums on DVE (2x tensor_tensor), each removes one PE tile
        feeds = [tile_view(acts.pop(0))] if acts else []
        for pr in prs:
            dsum = spool.tile([128, NB2, H, W], f16, tag="dsum",
                              name=f"dsum{o}_{pr[0]}")
            ka, kb = (pr[0] % 3, pr[1] % 3) if WIDE else (0, 0)
            nc.vector.tensor_add(
                dsum[:], ddo[:, slot[pr[0]], :, :, ka : ka + W],
                ddo[:, slot[pr[1]], :, :, kb : kb + W])
            feeds.append(tile_view(dsum))
        feeds.extend(slot_view(ddo, slot[t], (t % 3) if WIDE else 0)
                     for t in dve_taps if t not in in_pair)
        feeds.extend(tile_view(a) for a in acts)
        feeds.extend(tile_view(g) for g in pools)
        for i, f in enumerate(feeds):
            mm(o, f, last_of_o and i == len(feeds) - 1)


@functools.lru_cache(maxsize=4)
def _build_program(bench_iters=0):
    from contextlib import ExitStack

    import concourse.tile as tile
    from concourse import bacc, mybir

    f32 = mybir.dt.float32
    f16 = mybir.dt.float16

    nc = bacc.Bacc("TRN2", target_bir_lowering=False, debug=False)

    # host-padded f16 input, partition (h, c), free (u, y, x)
    xph_t = nc.dram_tensor("xph", (128, NB2 * SPP), f16, kind="ExternalInput")
    # wpack[:, :72] = wcols (w[o_g, p%64, tap]), [:, 72:144] = -wcols
    wpack_t = nc.dram_tensor("wpack", (128, 2 * O_PER * 9), f32, kind="ExternalInput")
    # mselh[p, (o*2+u)*32 + j] = 1.0 iff j == o*4 + 2u + p//64
    mselh_t = nc.dram_tensor("mselh", (128, O_PER * 2 * NSTAT), f16, kind="ExternalInput")
    # spack[:, :32] = osel, then -gamma col, beta col, eps col
    spack_t = nc.dram_tensor("spack", (NSTAT, NSTAT + 3), f32, kind="ExternalInput")
    out_t = nc.dram_tensor("out", (NSTAT, L), f32, kind="ExternalOutput")

    with tile.TileContext(nc) as tc, ExitStack() as ctx:
        consts = ctx.enter_context(tc.tile_pool(name="consts", bufs=1))
        apool = ctx.enter_context(tc.tile_pool(name="apool", bufs=D_BUFS))
        dpool = ctx.enter_context(tc.tile_pool(name="dpool", bufs=2))
        spool = ctx.enter_context(tc.tile_pool(name="spool", bufs=4))
        gpool = ctx.enter_context(tc.tile_pool(name="gpool", bufs=3))
        epool = ctx.enter_context(tc.tile_pool(name="epool", bufs=2))
        psum_main = ctx.enter_context(tc.tile_pool(name="psum_main", bufs=1, space="PSUM"))
        psum_stat = ctx.enter_context(tc.tile_pool(name="psum_stat", bufs=2, space="PSUM"))

        # ---- constants / inputs to SBUF ----
        wpack = consts.tile([128, 2 * O_PER * 9], f32)
        mselh = consts.tile([128, O_PER * 2 * NSTAT], f16)
        spack = consts.tile([NSTAT, NSTAT + 3], f32)
        nc.sync.dma_start(out=wpack[:], in_=wpack_t[:, :])
        nc.sync.dma_start(out=mselh[:], in_=mselh_t[:, :])
        nc.sync.dma_start(out=spack[:], in_=spack_t[:, :])
        wcols = wpack[:, 0 : O_PER * 9]
        nwcols = wpack[:, O_PER * 9 : 2 * O_PER * 9]
        osel = spack[:, 0:NSTAT]
        gcol = spack[:, NSTAT : NSTAT + 1]
        bcol = spack[:, NSTAT + 1 : NSTAT + 2]
        epscol = spack[:, NSTAT + 2 : NSTAT + 3]

        xph = consts.tile([128, NB2 * SPP], f16)
        for u in range(NB2):
            nc.sync.dma_start(
                out=xph[:, u * SPP : (u + 1) * SPP],
                in_=xph_t[:, u * SPP : (u + 1) * SPP],
            )
        xph4 = xph.rearrange("p (u a b) -> p u a b", u=NB2, a=HP, b=WP)

        # ---- main loop: S[o*4+b, l] accumulates over taps in two PSUM halves ----
        ps_big = psum_main.tile([NSTAT, 2 * 512], f32, name="ps_big")
        ps = [ps_big[:, h * 512 : h * 512 + 512] for h in range(2)]
        import contextlib

        loop_cm = (
            tc.For_i(0, bench_iters, 1) if bench_iters else contextlib.nullcontext()
        )
        with loop_cm:
            _emit_main(nc, tc, mybir, xph4, wcols, nwcols, mselh, apool, dpool,
                       spool, gpool, ps)

        # ---- epilogue: BN stats + normalize + relu ----
        sums = epool.tile([NSTAT, 1], f32, tag="small1")
        nc.vector.tensor_reduce(
            out=sums[:], in_=ps_big[:], axis=mybir.AxisListType.X,
            op=mybir.AluOpType.add,
        )
        sum_ps = psum_stat.tile([NSTAT, 1], f32, tag="statps")
        nc.tensor.matmul(sum_ps[:], lhsT=osel, rhs=sums[:], start=True, stop=True)
        # mcol = -mean
        mcol = epool.tile([NSTAT, 1], f32, tag="small2")
        nc.vector.tensor_scalar_mul(mcol[:], sum_ps[:], -1.0 / NPIX)

        # var*NPIX via Square((S - mean)) with centering folded into the bias
        scr = epool.tile([NSTAT, L], f32, tag="big")
        sqs = epool.tile([NSTAT, 1], f32, tag="small3")
        nc.scalar.activation(
            out=scr[:], in_=ps_big[:], func=mybir.ActivationFunctionType.Square,
            bias=mcol[:], scale=1.0, accum_out=sqs[:],
        )
        var_ps = psum_stat.tile([NSTAT, 1], f32, tag="statps2")
        nc.tensor.matmul(var_ps[:], lhsT=osel, rhs=sqs[:], start=True, stop=True)
        # std = sqrt(var/NPIX + eps); rinv = 1/std
        std32 = epool.tile([NSTAT, 1], f32, tag="small4")
        nc.scalar.activation(
            out=std32[:], in_=var_ps[:], func=mybir.ActivationFunctionType.Sqrt,
            bias=epscol, scale=1.0 / NPIX,
        )
        rinv = epool.tile([NSTAT, 1], f32, tag="small5")
        nc.vector.reciprocal(rinv[:], std32[:])
        # A = -gamma*rinv ; B = beta + mean*gamma*rinv = bcol + mcol*A
        acol = epool.tile([NSTAT, 1], f32, tag="small6")
        nc.vector.tensor_mul(acol[:], gcol, rinv[:])
        tmpc = epool.tile([NSTAT, 1], f32, tag="small7")
        nc.vector.tensor_mul(tmpc[:], mcol[:], acol[:])
        bias2 = epool.tile([NSTAT, 1], f32, tag="small8")
        nc.vector.tensor_add(bias2[:], bcol, tmpc[:])

        outf = epool.tile([NSTAT, L], f32, tag="outf")
        nc.scalar.activation(
            out=outf[:], in_=ps_big[:], func=mybir.ActivationFunctionType.Relu,
            bias=bias2[:], scale=acol[:],
        )
        nc.sync.dma_start(out=out_t[:, :], in_=outf[:])

    nc.compile()
    return nc


def _host_inputs(x, weight, gamma, beta):
    """Build the 8 per-core input maps."""
    x = np.ascontiguousarray(x, dtype=np.float32)
    weight = np.asarray(weight, dtype=np.float32)
    gamma = np.asarray(gamma, dtype=np.float32)
    beta = np.asarray(beta, dtype=np.float32)

    xph = np.zeros((128, NB2, HP, WP), dtype=np.float16)
    for b in range(B):
        h, u = b % 2, b // 2
        xph[h * 64 : (h + 1) * 64, u, PAD : PAD + H, PAD : PAD + W] = x[b].astype(
            np.float16
        )
    xph = np.ascontiguousarray(xph.reshape(128, NB2 * SPP))

    msel = np.zeros((128, O_PER * 2 * NSTAT), dtype=np.float32)
    for o in range(O_PER):
        for u in range(NB2):
            for p_half in range(2):
                j = o * 4 + 2 * u + p_half
                col = (o * 2 + u) * NSTAT + j
                msel[p_half * 64 : (p_half + 1) * 64, col] = 1.0
    osel = np.zeros((NSTAT, NSTAT), dtype=np.float32)
    for p in range(NSTAT):
        for m in range(NSTAT):
            if p // B == m // B:
                osel[p, m] = 1.0

    in_maps = []
    for core in range(NCORES):
        osl = slice(core * O_PER, (core + 1) * O_PER)
        w = weight[osl]  # [8, 64, 3, 3]
        # wcols[p, o*9+tap] = w[o, p%64, tap//3, tap%3]
        wc = w.reshape(O_PER, C, 9).transpose(1, 0, 2).reshape(C, O_PER * 9)
        wcols = np.concatenate([wc, wc], axis=0).astype(np.float32)  # [128, 72]
        wpack = np.concatenate([wcols, -wcols], axis=1)  # [128, 144]
        # gcol[p] = -gamma[o(p)] with o = p//4 (A = -gamma*rinv)
        gcol = np.repeat(-gamma[osl], B).reshape(NSTAT, 1).astype(np.float32)
        bcol = np.repeat(beta[osl], B).reshape(NSTAT, 1).astype(np.float32)
        epscol = np.full((NSTAT, 1), EPS, dtype=np.float32)
        spack = np.concatenate([osel, gcol, bcol, epscol], axis=1)  # [32, 35]
        in_maps.append(
            {
                "xph": xph,
                "wpack": np.ascontiguousarray(wpack),
                "mselh": msel.astype(np.float16),
                "spack": np.ascontiguousarray(spack),
            }
        )
    return in_maps


def _assemble(results):
    out = np.empty((B, O, H, W), dtype=np.float32)
    for core, res in enumerate(results):
        arr = res["out"].reshape(O_PER, B, H, W)  # row = o*4+b
        out[:, core * O_PER : (core + 1) * O_PER] = arr.transpose(1, 0, 2, 3)
    return out


def kernel(x, weight, gamma, beta, _trace=False):
    from concourse import bass_utils

    nc = _build_program()
    in_maps = _host_inputs(x, weight, gamma, beta)
    res = bass_utils.run_bass_kernel_spmd(
        nc, in_maps, core_ids=list(range(NCORES)), trace=_trace
    )
    out = _assemble(res.results)
    if _trace:
        return out, res
    return out


# revision 25
# speedup vs baseline: 1.2790x; 1.1824x over previous
"""AdderNet BasicBlock (Adder2D 3x3 + BatchNorm(train) + ReLU) on 8 TRN2 cores.

Problem: x[4,64,32,32], weight[64,64,3,3], gamma[64], beta[64] ->
    out[b,o,y,x] = relu(BN_train(-sum_{c,ky,kx} |x_pad[b,c,y+ky,x+kx] - w[o,c,ky,kx]|))

Sharding: output channels O=64 split 8 per core. BatchNorm stats are per-channel
over (B,H,W), so each core's 8 channels are fully self-contained: no collectives.

Per-core dataflow (all shapes hardcoded):
  - Host supplies XPH[128, 2*34*34] f16: padded input, partition p = (h, c) with
    h=p//64, c=p%64; free (u, y, x) holds batch b = 2*u + h. No on-chip
    conversion or memset; one strided f16 view per (o, tap).
  - ACT taps (kx==1 plus extras): one fused op, activation Abs with
    per-partition bias=-w. All DVE views then have even element offsets
    (kx in {0,2}), keeping 4-byte alignment for DVE 4x mode with no shifted
    input copy.
  - DVE taps: tensor_scalar subtract at 4x into slots of a per-o wide tile,
    then ONE merged sign-bit clear (tensor_scalar bitwise_and 0x7FFF on the
    u16 view, 4x) over all slots — fused sub+and / abs_max op1 variants are
    rejected by the walrus BIR verifier ("mismatch op0(arith) op1(bitwise)").
  - Presum: pairs of |diff| tiles get tensor_tensor-added on DVE (2x mode)
    before PE, trading ~1.1us DVE for ~0.9us PE per pair (PE is the
    bottleneck engine otherwise).
  - PE reduces over partitions, accumulating all 8 channels x 9 taps into one
    persistent PSUM tile S[32, 1024] via one-hot f16 selector lhsT
    (rows o*4 + 2u + h). f16 matmuls ~221.6 ns at N=512.
  - Epilogue from PSUM: mean via free-reduce + selector matmul; variance via
    ACT Square with bias=-mean (centering folded into the activation, no
    separate DVE pass) + accum_out; out = relu(S*A + B) with A = -gamma/std,
    B = beta + mean*gamma/std folded into the final ACT op; single DMA out.

kernel() is self-contained: builds the Bass program once, shards inputs on host,
runs via bass_utils.run_bass_kernel_spmd on cores 0..7, reassembles full output.
"""

import functools
import os

import numpy as np

B, C, O, H, W = 4, 64, 64, 32, 32
K, PAD = 3, 1
HP, WP = H + 2 * PAD, W + 2 * PAD  # 34, 34
L = H * W  # 1024
SPP = HP * WP  # 1156 padded spatial per batch
NCORES = 8
O_PER = O // NCORES  # 8
NB2 = B // 2  # bpairs
EPS = 1e-5
NSTAT = O_PER * B  # 32 rows of S
NPIX = B * L  # 4096 values per channel for BN stats

# ACT taps per core: kx==1 taps not taken by Pool, plus tap-0/tap-3 extras.
N_ACT_OPS = int(os.environ.get("KRN_ACT_OPS", "30"))
# Pool/gpsimd engine taps: measured ~40us+/op on HW (the Pool engine also
# serves DMA descriptor generation) — keep at 0.
N_GPS_OPS = int(os.environ.get("KRN_GPS_OPS", "0"))
N_PRESUM = int(os.environ.get("KRN_PRESUM", "8"))
D_BUFS = int(os.environ.get("KRN_D_BUFS", "8"))
MM_REPEAT = int(os.environ.get("KRN_MM_REPEAT", "1"))  # bench-only: scales PE work
# WIDE=1: DVE subs cover the full 34-wide padded rows (fully contiguous APs,
# guaranteed 4x eligibility) and the PE reads a strided window per kx.
WIDE = int(os.environ.get("KRN_WIDE", "0"))


def _schedule():
    """Per-o tap assignment: (act, dve, pool taps) and global presum pairs."""
    assert N_GPS_OPS <= 16
    # Pool poaches kx==1 taps (tap 4 of each o, then tap 1)
    pool_list = [(o, 4) for o in range(O_PER)] + [(o, 1) for o in range(O_PER)]
    pool_set = set(pool_list[:N_GPS_OPS])
    # ACT: remaining kx==1 taps + extras (tap 0, then tap 3) up to N_ACT_OPS
    extras = [(o, 0) for o in range(O_PER)] + [(o, 3) for o in range(O_PER)]
    n_base = 24 - len(pool_set)
    n_extra = max(0, min(N_ACT_OPS - n_base, len(extras)))
    act_set = {(o, t) for o in range(O_PER) for t in (1, 4, 7)
               if (o, t) not in pool_set}
    act_set |= set(extras[:n_extra])
    plans = []
    for o in range(O_PER):
        act = [t for t in (0, 1, 3, 4, 7) if (o, t) in act_set]
        pool = [t for t in (4, 1) if (o, t) in pool_set]
        dve = [t for t in range(9) if t not in act and t not in pool]
        plans.append((act, dve, pool))
    # presum pair priority: (3,5) for each o, then (6,8), then (0,2) if on DVE
    pairs = []
    for pr in [(3, 5), (6, 8), (0, 2)]:
        for o in range(O_PER):
            if pr[0] in plans[o][1] and pr[1] in plans[o][1]:
                pairs.append((o, pr))
    return plans, pairs[: max(0, min(N_PRESUM, len(pairs)))]


def _emit_main(nc, tc, mybir, xph4, wcols, nwcols, mselh, apool, dpool, spool,
               gpool, ps):
    from concourse import mybir as _mb

    f16 = _mb.dt.float16
    u16 = _mb.dt.uint16
    plans, pairs = _schedule()
    pair_of = {o: [] for o in range(O_PER)}
    for o, pr in pairs:
        pair_of[o].append(pr)

    first = [True, True]
    WD = WP if WIDE else W

    def mm(o, view_fn, last):
        # view_fn(u, half) -> [128, 16, W] rhs window (512 cols)
        for rep in range(MM_REPEAT):
            for u in range(NB2):
                for half in range(2):
                    nc.tensor.matmul(
                        ps[half][:, :],
                        lhsT=mselh[:, (o * 2 + u) * NSTAT : (o * 2 + u + 1) * NSTAT],
                        rhs=view_fn(u, half),
                        start=first[half],
                        stop=(last and rep == MM_REPEAT - 1 and u == NB2 - 1),
                    )
                    first[half] = False

    def tile_view(t):
        return lambda u, half: t[:, u, half * 16 : half * 16 + 16, 0:W]

    def slot_view(t, s, kx0):
        return lambda u, half: t[:, s, u, half * 16 : half * 16 + 16, kx0 : kx0 + W]

    for o in range(O_PER):
        act_taps, dve_taps, pool_taps = plans[o]
        n = len(dve_taps)
        prs = pair_of[o]
        in_pair = {t for pr in prs for t in pr}
        last_of_o = o == O_PER - 1

        # Pool taps (slow 1x engine, otherwise idle): subtract on Pool, then a
        # 4x sign-clear on DVE (walrus rejects bitwise tensor_scalar on Pool)
        pools = []
        for t in pool_taps:
            ky, kx = t // 3, t % 3
            idx = o * 9 + t
            g1 = gpool.tile([128, NB2, H, W], f16, tag="g1", name=f"g1_{idx}")
            nc.gpsimd.tensor_scalar_sub(
                g1[:], xph4[:, :, ky : ky + H, kx : kx + W],
                wcols[:, idx : idx + 1])
            g2 = gpool.tile([128, NB2, H, W], f16, tag="g2", name=f"g2_{idx}")
            nc.vector.tensor_scalar(
                out=g2[:].bitcast(u16), in0=g1[:].bitcast(u16),
                scalar1=0x7FFF, scalar2=None, op0=_mb.AluOpType.bitwise_and)
            pools.append(g2)

        # ACT taps: one fused |x - w| op each
        acts = []
        for t in act_taps:
            ky, kx = t // 3, t % 3
            idx = o * 9 + t
            da = apool.tile([128, NB2, H, W], f16, tag="da", name=f"da{idx}")
            nc.scalar.activation(
                out=da[:], in_=xph4[:, :, ky : ky + H, kx : kx + W],
                func=_mb.ActivationFunctionType.Abs,
                bias=nwcols[:, idx : idx + 1], scale=1.0,
            )
            acts.append(da)

        # DVE taps: subs into slots of one wide tile, one merged AND over all
        dd = dpool.tile([128, 6, NB2, H, WD], f16, tag="dd", name=f"dd{o}")
        ddo = dpool.tile([128, 6, NB2, H, WD], f16, tag="ddo", name=f"ddo{o}")
        slot = {t: i for i, t in enumerate(dve_taps)}
        for t in dve_taps:
            ky, kx = t // 3, t % 3
            idx = o * 9 + t
            src = (xph4[:, :, ky : ky + H, :] if WIDE
                   else xph4[:, :, ky : ky + H, kx : kx + W])
            nc.vector.tensor_scalar_sub(dd[:, slot[t]], src,
                                        wcols[:, idx : idx + 1])
        nc.vector.tensor_scalar(
            out=ddo[:, 0:n].bitcast(u16), in0=dd[:, 0:n].bitcast(u16),
            scalar1=0x7FFF, scalar2=None, op0=_mb.AluOpType.bitwise_and,
        )

        # presums on DVE (2x tensor_tensor), each removes one PE tile
        feeds = [tile_view(acts.pop(0))] if acts else []
        for pr in prs:
            dsum = spool.tile([128, NB2, H, W], f16, tag="dsum",
                              name=f"dsum{o}_{pr[0]}")
            ka, kb = (pr[0] % 3, pr[1] % 3) if WIDE else (0, 0)
            nc.vector.tensor_add(
                dsum[:], ddo[:, slot[pr[0]], :, :, ka : ka + W],
                ddo[:, slot[pr[1]], :, :, kb : kb + W])
            feeds.append(tile_view(dsum))
        feeds.extend(slot_view(ddo, slot[t], (t % 3) if WIDE else 0)
                     for t in dve_taps if t not in in_pair)
        feeds.extend(tile_view(a) for a in acts)
        feeds.extend(tile_view(g) for g in pools)
        for i, f in enumerate(feeds):
            mm(o, f, last_of_o and i == len(feeds) - 1)


@functools.lru_cache(maxsize=4)
def _build_program(bench_iters=0):
    from contextlib import ExitStack

    import concourse.tile as tile
    from concourse import bacc, mybir

    f32 = mybir.dt.float32
    f16 = mybir.dt.float16

    nc = bacc.Bacc("TRN2", target_bir_lowering=False, debug=False)

    # host-padded f16 input, partition (h, c), free (u, y, x)
    xph_t = nc.dram_tensor("xph", (128, NB2 * SPP), f16, kind="ExternalInput")
    # wpack[:, :72] = wcols (w[o_g, p%64, tap]), [:, 72:144] = -wcols
    wpack_t = nc.dram_tensor("wpack", (128, 2 * O_PER * 9), f32, kind="ExternalInput")
    # mselh[p, (o*2+u)*32 + j] = 1.0 iff j == o*4 + 2u + p//64
    mselh_t = nc.dram_tensor("mselh", (128, O_PER * 2 * NSTAT), f16, kind="ExternalInput")
    # spack[:, :32] = osel, then -gamma col, beta col, eps col
    spack_t = nc.dram_tensor("spack", (NSTAT, NSTAT + 3), f32, kind="ExternalInput")
    out_t = nc.dram_tensor("out", (NSTAT, L), f32, kind="ExternalOutput")

    with tile.TileContext(nc) as tc, ExitStack() as ctx:
        consts = ctx.enter_context(tc.tile_pool(name="consts", bufs=1))
        apool = ctx.enter_context(tc.tile_pool(name="apool", bufs=D_BUFS))
        dpool = ctx.enter_context(tc.tile_pool(name="dpool", bufs=2))
        spool = ctx.enter_context(tc.tile_pool(name="spool", bufs=4))
        gpool = ctx.enter_context(tc.tile_pool(name="gpool", bufs=3))
        epool = ctx.enter_context(tc.tile_pool(name="epool", bufs=2))
        psum_main = ctx.enter_context(tc.tile_pool(name="psum_main", bufs=1, space="PSUM"))
        psum_stat = ctx.enter_context(tc.tile_pool(name="psum_stat", bufs=2, space="PSUM"))

        # ---- constants / inputs to SBUF ----
        wpack = consts.tile([128, 2 * O_PER * 9], f32)
        mselh = consts.tile([128, O_PER * 2 * NSTAT], f16)
        spack = consts.tile([NSTAT, NSTAT + 3], f32)
        nc.sync.dma_start(out=wpack[:], in_=wpack_t[:, :])
        nc.sync.dma_start(out=mselh[:], in_=mselh_t[:, :])
        nc.sync.dma_start(out=spack[:], in_=spack_t[:, :])
        wcols = wpack[:, 0 : O_PER * 9]
        nwcols = wpack[:, O_PER * 9 : 2 * O_PER * 9]
        osel = spack[:, 0:NSTAT]
        gcol = spack[:, NSTAT : NSTAT + 1]
        bcol = spack[:, NSTAT + 1 : NSTAT + 2]
        epscol = spack[:, NSTAT + 2 : NSTAT + 3]

        xph = consts.tile([128, NB2 * SPP], f16)
        for u in range(NB2):
            nc.sync.dma_start(
                out=xph[:, u * SPP : (u + 1) * SPP],
                in_=xph_t[:, u * SPP : (u + 1) * SPP],
            )
        xph4 = xph.rearrange("p (u a b) -> p u a b", u=NB2, a=HP, b=WP)

        # ---- main loop: S[o*4+b, l] accumulates over taps in two PSUM halves ----
        ps_big = psum_main.tile([NSTAT, 2 * 512], f32, name="ps_big")
        ps = [ps_big[:, h * 512 : h * 512 + 512] for h in range(2)]
        import contextlib

        loop_cm = (
            tc.For_i(0, bench_iters, 1) if bench_iters else contextlib.nullcontext()
        )
        with loop_cm:
            _emit_main(nc, tc, mybir, xph4, wcols, nwcols, mselh, apool, dpool,
                       spool, gpool, ps)

        # ---- epilogue: BN stats + normalize + relu ----
        sums = epool.tile([NSTAT, 1], f32, tag="small1")
        nc.vector.tensor_reduce(
            out=sums[:], in_=ps_big[:], axis=mybir.AxisListType.X,
            op=mybir.AluOpType.add,
        )
        sum_ps = psum_stat.tile([NSTAT, 1], f32, tag="statps")
        nc.tensor.matmul(sum_ps[:], lhsT=osel, rhs=sums[:], start=True, stop=True)
        # mcol = -mean
        mcol = epool.tile([NSTAT, 1], f32, tag="small2")
        nc.vector.tensor_scalar_mul(mcol[:], sum_ps[:], -1.0 / NPIX)

        # var*NPIX via Square((S - mean)) with centering folded into the bias
        scr = epool.tile([NSTAT, L], f32, tag="big")
        sqs = epool.tile([NSTAT, 1], f32, tag="small3")
        nc.scalar.activation(
            out=scr[:], in_=ps_big[:], func=mybir.ActivationFunctionType.Square,
            bias=mcol[:], scale=1.0, accum_out=sqs[:],
        )
        var_ps = psum_stat.tile([NSTAT, 1], f32, tag="statps2")
        nc.tensor.matmul(var_ps[:], lhsT=osel, rhs=sqs[:], start=True, stop=True)
        # std = sqrt(var/NPIX + eps); rinv = 1/std
        std32 = epool.tile([NSTAT, 1], f32, tag="small4")
        nc.scalar.activation(
            out=std32[:], in_=var_ps[:], func=mybir.ActivationFunctionType.Sqrt,
            bias=epscol, scale=1.0 / NPIX,
        )
        rinv = epool.tile([NSTAT, 1], f32, tag="small5")
        nc.vector.reciprocal(rinv[:], std32[:])
        # A = -gamma*rinv ; B = beta + mean*gamma*rinv = bcol + mcol*A
        acol = epool.tile([NSTAT, 1], f32, tag="small6")
        nc.vector.tensor_mul(acol[:], gcol, rinv[:])
        tmpc = epool.tile([NSTAT, 1], f32, tag="small7")
        nc.vector.tensor_mul(tmpc[:], mcol[:], acol[:])
        bias2 = epool.tile([NSTAT, 1], f32, tag="small8")
        nc.vector.tensor_add(bias2[:], bcol, tmpc[:])

        outf = epool.tile([NSTAT, L], f32, tag="outf")
        nc.scalar.activation(
            out=outf[:], in_=ps_big[:], func=mybir.ActivationFunctionType.Relu,
            bias=bias2[:], scale=acol[:],
        )
        nc.sync.dma_start(out=out_t[:, :], in_=outf[:])

    nc.compile()
    return nc


def _host_inputs(x, weight, gamma, beta):
    """Build the 8 per-core input maps."""
    x = np.ascontiguousarray(x, dtype=np.float32)
    weight = np.asarray(weight, dtype=np.float32)
    gamma = np.asarray(gamma, dtype=np.float32)
    beta = np.asarray(beta, dtype=np.float32)

    xph = np.zeros((128, NB2, HP, WP), dtype=np.float16)
    for b in range(B):
        h, u = b % 2, b // 2
        xph[h * 64 : (h + 1) * 64, u, PAD : PAD + H, PAD : PAD + W] = x[b].astype(
            np.float16
        )
    xph = np.ascontiguousarray(xph.reshape(128, NB2 * SPP))

    msel = np.zeros((128, O_PER * 2 * NSTAT), dtype=np.float32)
    for o in range(O_PER):
        for u in range(NB2):
            for p_half in range(2):
                j = o * 4 + 2 * u + p_half
                col = (o * 2 + u) * NSTAT + j
                msel[p_half * 64 : (p_half + 1) * 64, col] = 1.0
    osel = np.zeros((NSTAT, NSTAT), dtype=np.float32)
    for p in range(NSTAT):
        for m in range(NSTAT):
            if p // B == m // B:
                osel[p, m] = 1.0

    in_maps = []
    for core in range(NCORES):
        osl = slice(core * O_PER, (core + 1) * O_PER)
        w = weight[osl]  # [8, 64, 3, 3]
        # wcols[p, o*9+tap] = w[o, p%64, tap//3, tap%3]
        wc = w.reshape(O_PER, C, 9).transpose(1, 0, 2).reshape(C, O_PER * 9)
        wcols = np.concatenate([wc, wc], axis=0).astype(np.float32)  # [128, 72]
        wpack = np.concatenate([wcols, -wcols], axis=1)  # [128, 144]
        # gcol[p] = -gamma[o(p)] with o = p//4 (A = -gamma*rinv)
        gcol = np.repeat(-gamma[osl], B).reshape(NSTAT, 1).astype(np.float32)
        bcol = np.repeat(beta[osl], B).reshape(NSTAT, 1).astype(np.float32)
        epscol = np.full((NSTAT, 1), EPS, dtype=np.float32)
        spack = np.concatenate([osel, gcol, bcol, epscol], axis=1)  # [32, 35]
        in_maps.append(
            {
                "xph": xph,
                "wpack": np.ascontiguousarray(wpack),
                "mselh": msel.astype(np.float16),
                "spack": np.ascontiguousarray(spack),
            }
        )
    return in_maps


def _assemble(results):
    out = np.empty((B, O, H, W), dtype=np.float32)
    for core, res in enumerate(results):
        arr = res["out"].reshape(O_PER, B, H, W)  # row = o*4+b
        out[:, core * O_PER : (core + 1) * O_PER] = arr.transpose(1, 0, 2, 3)
    return out


def kernel(x, weight, gamma, beta, _trace=False):
    from concourse import bass_utils

    nc = _build_program()
    in_maps = _host_inputs(x, weight, gamma, beta)
    res = bass_utils.run_bass_kernel_spmd(
        nc, in_maps, core_ids=list(range(NCORES)), trace=_trace
    )
    out = _assemble(res.results)
    if _trace:
        return out, res
    return out
